# revision 1
# baseline (speedup 1.0000x reference)
"""Convex multi-head attention kernel for Trainium2 (8 NeuronCores).

Problem: out = combine_heads( convex_softmax(Q @ K^T) @ V ) where
  X_proj = x @ W + b;  Q/K/V = split_heads(X_proj * d_q / d_k / d_v)
  convex_softmax(z) = relu(exp(clip(z,-15,15) - R) + LAM*clip(z)) / row_sum

Sharding (no collectives needed): core c -> batch b = c // 4, heads
4*(c%4) .. 4*(c%4)+3 (256 contiguous columns of the output). Each core
computes its full [2048, 256] output slice; host concatenates.

Math restructuring used on-device (per score element z):
  * numerator  n = relu(exp(z_c - R) + LAM*z_c), z_c = clip(z, -15, 15).
    Scaling by 1/LAM cancels in the normalization, so use
      n' = exp(m - R - ln(LAM)) + m   with  m = clip(z, Z0, 15),
    where Z0 is the root of exp(m - R) + LAM*m = 0 (Z0 ~ -1.1569 > -15).
    For z <= Z0 the true numerator is 0 and n'(Z0) = 0 exactly, so the
    relu AND the lower clip fold into the clamp bound.  One DVE dual-op
    tensor_scalar (min 15, max Z0) + one ACT exp per element.
  * n' @ V = E @ V + M @ V (matmul linearity) avoids materializing E+M.
  * V gets an extra ones-column so the second matmul also produces the
    row-sums; division by the row-sum happens on the [S, 64] output.
  * All matmuls run as float32r (full fp32 data, ~bf16 PE throughput).
  * Attention is computed fully transposed (scores^T[t,s]) so the second
    matmul consumes E^T/M^T directly as the moving operand.
"""

import math
import os
import sys

import numpy as np

sys.path.insert(0, "/opt/trn_rl_repo")

# ---------------- problem constants (hardcoded per spec) ----------------
B = 2
S = 2048
D_MODEL = 1024
NUM_HEADS = 16
HEAD_DIM = 64
R = 1.0
LAM = 0.1
CLIP_MAX = 15.0
CLIP_MIN = -15.0

N_CORES = 8
HPC = NUM_HEADS // (N_CORES // B)  # heads per core = 4
DS = HPC * HEAD_DIM                # per-core d-slice = 256
KT = D_MODEL // 128                # 8 contraction tiles
ST = S // 128                      # 16 sequence tiles
VW = HEAD_DIM + 1                  # 65: V columns + ones column

# exp argument bias: exp(m - R - ln(LAM)) = (1/LAM) * exp(m - R)
C_EXP = -R - math.log(LAM)

def _solve_z0() -> float:
    # root of g(m) = exp(m - R) + LAM * m  (monotone increasing)
    lo, hi = -10.0, 10.0
    for _ in range(200):
        mid = 0.5 * (lo + hi)
        if math.exp(mid - R) + LAM * mid > 0.0:
            hi = mid
        else:
            lo = mid
    return 0.5 * (lo + hi)

Z0 = _solve_z0()
assert Z0 > CLIP_MIN + 1e-6, "relu-fold requires Z0 > CLIP_MIN"

_NC_CACHE = {}


def _build_nc():
    """Build (once) the single-core Bass/Tile program shared by all cores."""
    if "nc" in _NC_CACHE:
        return _NC_CACHE["nc"]

    from contextlib import ExitStack

    import concourse.bass as bass
    import concourse.mybir as mybir
    import concourse.tile as tile
    from concourse import bacc
    from concourse.masks import make_identity

    f32 = mybir.dt.float32
    f32r = mybir.dt.float32r
    Alu = mybir.AluOpType
    Act = mybir.ActivationFunctionType

    nc = bacc.Bacc("TRN2", target_bir_lowering=False, debug=False)

    x_d = nc.dram_tensor("x", [S, D_MODEL], f32, kind="ExternalInput")
    w_d = nc.dram_tensor("w", [D_MODEL, DS], f32, kind="ExternalInput")
    wv_d = nc.dram_tensor("wv", [D_MODEL, DS], f32, kind="ExternalInput")
    # [128, 2] per-partition vectors per d-tile: dsc = d_q*d_k, ab = dsc*b, bb = b
    dsc_d = nc.dram_tensor("dsc", [128, 2], f32, kind="ExternalInput")
    ab_d = nc.dram_tensor("ab", [128, 2], f32, kind="ExternalInput")
    bb_d = nc.dram_tensor("bb", [128, 2], f32, kind="ExternalInput")
    bv_d = nc.dram_tensor("bv", [DS], f32, kind="ExternalInput")
    out_d = nc.dram_tensor("out", [S, DS], f32, kind="ExternalOutput")

    def r32(ap):
        return ap.bitcast(f32r)

    with tile.TileContext(nc) as tc, ExitStack() as ctx:
        persist = ctx.enter_context(tc.tile_pool(name="persist", bufs=1))

        ident = persist.tile([128, 128], f32, tag="ident")
        make_identity(nc, ident)

        cexp_sb = persist.tile([128, 1], f32, tag="cexp")
        nc.vector.memset(cexp_sb, C_EXP)

        dsc_sb = persist.tile([128, 2], f32, tag="dsc")
        nc.sync.dma_start(out=dsc_sb, in_=dsc_d.ap())
        ab_sb = persist.tile([128, 2], f32, tag="ab")
        nc.sync.dma_start(out=ab_sb, in_=ab_d.ap())
        bb_sb = persist.tile([128, 2], f32, tag="bb")
        nc.sync.dma_start(out=bb_sb, in_=bb_d.ap())

        # broadcast (d_v * b) slice across all partitions: [128, DS]
        bv_bc = persist.tile([128, DS], f32, tag="bvbc")
        bv_ap = bv_d.ap()
        bv_bcast = bass.AP(tensor=bv_ap.tensor, offset=bv_ap.offset,
                           ap=[[0, 128]] + list(bv_ap.ap))
        nc.sync.dma_start(out=bv_bc, in_=bv_bcast)

        w_sb = persist.tile([128, KT, DS], f32r, tag="w")
        wv_sb = persist.tile([128, KT, DS], f32r, tag="wv")
        for kt in range(KT):
            nc.sync.dma_start(out=w_sb[:, kt, :], in_=r32(w_d[kt * 128:(kt + 1) * 128, :]))
            nc.sync.dma_start(out=wv_sb[:, kt, :], in_=r32(wv_d[kt * 128:(kt + 1) * 128, :]))

        # A = dsc * X_proj^T-slice (+dsc*b), B = X_proj^T-slice (+b): [128, 2, S]
        A_sb = persist.tile([128, 2, S], f32r, tag="A")
        B_sb = persist.tile([128, 2, S], f32r, tag="B")
        # V (+ones col) in natural layout: [128(t within tile), ST, 4*VW]
        V_sb = persist.tile([128, ST, HPC * VW], f32r, tag="V")
        for h in range(HPC):
            nc.vector.memset(V_sb[:, :, h * VW + HEAD_DIM].bitcast(f32), 1.0)

        # ---------------- phase 0: x^T, X_proj^T (A/B), V ----------------
        with tc.tile_pool(name="xT", bufs=1) as xtp, \
             tc.tile_pool(name="xnat", bufs=8) as xnp, \
             tc.tile_pool(name="ptr", bufs=2, space="PSUM") as ptrp, \
             tc.tile_pool(name="pxp", bufs=2, space="PSUM") as pxpp, \
             tc.tile_pool(name="pv", bufs=2, space="PSUM") as pvp:
            xT = xtp.tile([128, KT, S], f32r)  # x^T: [k within tile, kt, s]

            for sg in range(4):  # groups of 512 s-rows
                xnat = []
                for j in range(4):
                    t = xnp.tile([128, D_MODEL], f32, tag="xn", name=f"xn{sg}_{j}")
                    st = sg * 4 + j
                    nc.sync.dma_start(out=t, in_=x_d[st * 128:(st + 1) * 128, :])
                    xnat.append(t)
                for ktg in range(4):  # pairs of k-tiles
                    ptr = ptrp.tile([128, 2, 512], f32, tag="ptr")
                    for i in range(2):
                        kt = ktg * 2 + i
                        for j in range(4):
                            nc.tensor.transpose(
                                ptr[:, i, j * 128:(j + 1) * 128],
                                xnat[j][:, kt * 128:(kt + 1) * 128],
                                ident,
                            )
                    for i in range(2):
                        kt = ktg * 2 + i
                        dst = xT[:, kt, sg * 512:(sg + 1) * 512]
                        if i == 0:
                            nc.scalar.copy(dst, ptr[:, i, :])
                        else:
                            nc.vector.tensor_copy(dst, ptr[:, i, :])

                # X_proj^T for this s-block: out rows = our 256 d-cols
                for dt in range(2):
                    pxp = pxpp.tile([128, 512], f32, tag="pxp")
                    for kt in range(KT):
                        nc.tensor.matmul(
                            pxp,
                            w_sb[:, kt, dt * 128:(dt + 1) * 128],
                            xT[:, kt, sg * 512:(sg + 1) * 512],
                            start=(kt == 0),
                            stop=(kt == KT - 1),
                        )
                    nc.scalar.activation(
                        A_sb[:, dt, sg * 512:(sg + 1) * 512], pxp,
                        Act.Identity, bias=ab_sb[:, dt:dt + 1],
                        scale=dsc_sb[:, dt:dt + 1],
                    )
                    nc.scalar.activation(
                        B_sb[:, dt, sg * 512:(sg + 1) * 512], pxp,
                        Act.Identity, bias=bb_sb[:, dt:dt + 1], scale=1.0,
                    )

                # V rows for this s-block (4 t-tiles)
                for j in range(4):
                    st = sg * 4 + j
                    pv = pvp.tile([128, DS], f32, tag="pv")
                    for kt in range(KT):
                        nc.tensor.matmul(
                            pv,
                            xT[:, kt, st * 128:(st + 1) * 128],
                            wv_sb[:, kt, :],
                            start=(kt == 0),
                            stop=(kt == KT - 1),
                        )
                    dst = V_sb[:, st, :].rearrange("p (h c) -> p h c", h=HPC)[:, :, 0:HEAD_DIM]
                    nc.vector.tensor_add(
                        dst,
                        pv.rearrange("p (h c) -> p h c", h=HPC),
                        bv_bc.rearrange("p (h c) -> p h c", h=HPC),
                    )

        # ---------------- main attention loop ----------------
        with tc.tile_pool(name="zp", bufs=2, space="PSUM") as zp, \
             tc.tile_pool(name="pop", bufs=2, space="PSUM") as pop, \
             tc.tile_pool(name="mp", bufs=6) as mp, \
             tc.tile_pool(name="ep", bufs=6) as ep, \
             tc.tile_pool(name="op", bufs=3) as op, \
             tc.tile_pool(name="outp", bufs=4) as outp, \
             tc.tile_pool(name="recp", bufs=4) as recp:
            for hp in range(2):        # head pair
                for sh in range(2):    # s-half (1024 query columns)
                    po = [pop.tile([VW, 1024], f32, tag="po", name=f"po{hp}_{sh}_{i}") for i in range(2)]
                    for tt in range(ST):
                        for h2 in range(2):
                            ha = hp * 2 + h2
                            dt, r0 = ha // 2, 64 * (ha % 2)
                            z_t = zp.tile([128, 1024], f32, tag="zslot")
                            for nb in range(2):
                                nc.tensor.matmul(
                                    z_t[:, nb * 512:(nb + 1) * 512],
                                    A_sb[r0:r0 + 64, dt, tt * 128:(tt + 1) * 128],
                                    B_sb[r0:r0 + 64, dt,
                                         sh * 1024 + nb * 512:sh * 1024 + (nb + 1) * 512],
                                    start=True, stop=True,
                                )
                            m_t = mp.tile([128, 1024], f32r, tag="m")
                            nc.vector.tensor_scalar(
                                out=m_t, in0=z_t,
                                scalar1=CLIP_MAX, scalar2=Z0,
                                op0=Alu.min, op1=Alu.max,
                            )
                            e_t = ep.tile([128, 1024], f32r, tag="e")
                            nc.scalar.activation(e_t, m_t.bitcast(f32), Act.Exp,
                                                 bias=cexp_sb[:, 0:1], scale=1.0)
                            for si, src in enumerate((e_t, m_t)):
                                for nb in range(2):
                                    nc.tensor.matmul(
                                        po[h2][:, nb * 512:(nb + 1) * 512],
                                        V_sb[:, tt, ha * VW:(ha + 1) * VW],
                                        src[:, nb * 512:(nb + 1) * 512],
                                        start=(tt == 0 and si == 0),
                                        stop=(tt == ST - 1 and si == 1),
                                    )
                    # finalize: transpose out^T -> natural, divide by row-sum
                    o_sb = []
                    for h2 in range(2):
                        t = op.tile([VW, 1024], f32, tag="o", name=f"o{hp}_{sh}_{h2}")
                        nc.scalar.copy(t, po[h2])
                        o_sb.append(t)
                    for st in range(8):
                        pon = zp.tile([128, 2 * VW], f32, tag="zslot")
                        rec = recp.tile([128, 2], f32, tag="rec")
                        out_sb = outp.tile([128, 128], f32, tag="out")
                        for h2 in range(2):
                            nc.tensor.transpose(
                                pon[:, h2 * VW:(h2 + 1) * VW],
                                o_sb[h2][:, st * 128:(st + 1) * 128],
                                ident[0:VW, 0:VW],
                            )
                        nc.vector.reciprocal(
                            rec,
                            pon.rearrange("p (h c) -> p h c", h=2)[:, :, HEAD_DIM],
                        )
                        nc.scalar.activation(
                            out_sb[:, 0:64], pon[:, 0:HEAD_DIM],
                            Act.Identity, bias=0.0, scale=rec[:, 0:1],
                        )
                        nc.vector.tensor_scalar(
                            out=out_sb[:, 64:128],
                            in0=pon[:, VW:VW + HEAD_DIM],
                            scalar1=rec[:, 1:2], scalar2=None,
                            op0=Alu.mult,
                        )
                        nc.sync.dma_start(
                            out=out_d[sh * 1024 + st * 128:sh * 1024 + (st + 1) * 128,
                                      hp * 128:(hp + 1) * 128],
                            in_=out_sb,
                        )

    nc.compile()
    _NC_CACHE["nc"] = nc
    return nc


def kernel(x, W, b, d_q, d_k, d_v):
    """Full-input entry point: shards across 8 NeuronCores, returns [B,S,D]."""
    from concourse.bass_utils import run_bass_kernel_spmd

    nc = _build_nc()

    x = np.asarray(x, dtype=np.float32)
    W = np.asarray(W, dtype=np.float32)
    b = np.asarray(b, dtype=np.float32)
    d_q = np.asarray(d_q, dtype=np.float32)
    d_k = np.asarray(d_k, dtype=np.float32)
    d_v = np.asarray(d_v, dtype=np.float32)

    Wv = W * d_v[None, :]
    dsc = d_q * d_k
    ab_full = dsc * b
    bv_full = d_v * b

    in_maps = []
    for c in range(N_CORES):
        bi = c // (N_CORES // B)
        g = c % (N_CORES // B)
        sl = slice(g * DS, (g + 1) * DS)
        in_maps.append({
            "x": np.ascontiguousarray(x[bi]),
            "w": np.ascontiguousarray(W[:, sl]),
            "wv": np.ascontiguousarray(Wv[:, sl]),
            "dsc": np.ascontiguousarray(dsc[sl].reshape(2, 128).T),
            "ab": np.ascontiguousarray(ab_full[sl].reshape(2, 128).T),
            "bb": np.ascontiguousarray(b[sl].reshape(2, 128).T),
            "bv": np.ascontiguousarray(bv_full[sl]),
        })

    trace = os.environ.get("KERNEL_TRACE", "0") == "1"
    res = run_bass_kernel_spmd(nc, in_maps, list(range(N_CORES)), trace=trace)
    if trace:
        _NC_CACHE["last_results"] = res

    out = np.empty((B, S, D_MODEL), dtype=np.float32)
    for c in range(N_CORES):
        bi = c // (N_CORES // B)
        g = c % (N_CORES // B)
        out[bi, :, g * DS:(g + 1) * DS] = res.results[c]["out"]
    return out



# revision 5
# speedup vs baseline: 6.8145x; 6.8145x over previous
"""Convex multi-head attention kernel for Trainium2 (8 NeuronCores).

Problem: out = combine_heads( convex_softmax(Q @ K^T) @ V ) where
  X_proj = x @ W + b;  Q/K/V = split_heads(X_proj * d_q / d_k / d_v)
  convex_softmax(z) = relu(exp(clip(z,-15,15) - R) + LAM*clip(z)) / row_sum

Sharding: core c -> batch b = c // 4, heads 4*(c%4) .. 4*(c%4)+3 (256
contiguous columns of the output). Each core computes its full
[2048, 256] output slice; host concatenates.

Host<->device traffic is the wall-clock bottleneck (axon link: ~130 ms
fixed per transfer + ~85 MB/s), so the per-call protocol is minimized:
  * ONE packed fp16 input blob per core (x shard [512,1024] + W slice
    [1024,256] + derived per-slice vectors), 12 MB total in one put.
  * x is deduplicated on device: each core uploads a distinct quarter of
    its batch's rows; an AllGather over each batch quad (replica groups
    [[0..3],[4..7]]) assembles the full [2048,1024] x in DRAM.
  * W*d_v, dsc*X_proj etc. are derived on device.
  * fp16 output (one 8 MB get); host upcasts to fp32.
  * The jitted PJRT executable is cached across calls; donated zero
    output buffers are replaced by persistent on-device dummies (the
    kernel writes every output element).

Math restructuring on-device (per score element z):
  * numerator  n = relu(exp(z_c - R) + LAM*z_c), z_c = clip(z, -15, 15).
    Scaling by 1/LAM cancels in the normalization, so use
      n' = exp(m - R - ln(LAM)) + m   with  m = clip(z, Z0, 15),
    where Z0 is the root of exp(m - R) + LAM*m = 0 (Z0 ~ -1.1569 > -15).
    For z <= Z0 the true numerator is 0 and n'(Z0) = 0 exactly, so the
    relu AND the lower clip fold into the clamp bound.  One DVE dual-op
    tensor_scalar (min 15, max Z0) + one ACT exp per element.
  * n' @ V = E @ V + M @ V (matmul linearity) avoids materializing E+M.
  * V gets an extra ones-column so the second matmul also produces the
    row-sums; division by the row-sum happens on the [S, 64] output.
  * All matmuls run as float32r (full fp32 data, ~bf16 PE throughput).
  * Attention is computed fully transposed (scores^T[t,s]) so the second
    matmul consumes E^T/M^T directly as the moving operand.
"""

import math
import sys

import numpy as np

sys.path.insert(0, "/opt/trn_rl_repo")

# ---------------- problem constants (hardcoded per spec) ----------------
B = 2
S = 2048
D_MODEL = 1024
NUM_HEADS = 16
HEAD_DIM = 64
R = 1.0
LAM = 0.1
CLIP_MAX = 15.0
CLIP_MIN = -15.0

N_CORES = 8
HPC = NUM_HEADS // (N_CORES // B)  # heads per core = 4
DS = HPC * HEAD_DIM                # per-core d-slice = 256
KT = D_MODEL // 128                # 8 contraction tiles
ST = S // 128                      # 16 sequence tiles
VW = HEAD_DIM + 1                  # 65: V columns + ones column
XROWS = S // 4                     # 512 x rows uploaded per core

# blob layout (fp16 elements)
XOFF = 0                           # x shard [512, 1024]
WOFF = XROWS * D_MODEL             # W slice [1024, 256] row-major flat
VOFF = WOFF + D_MODEL * DS         # vec row: dsc|ab|bb (3*256), then dv, bv
DVOFF = VOFF + 3 * DS
BVOFF = DVOFF + DS
BLOB_ELEMS = (XROWS + DS + 2) * D_MODEL  # 770 rows of 1024

# exp argument bias: exp(m - R - ln(LAM)) = (1/LAM) * exp(m - R)
C_EXP = -R - math.log(LAM)

def _solve_z0() -> float:
    # root of g(m) = exp(m - R) + LAM * m  (monotone increasing)
    lo, hi = -10.0, 10.0
    for _ in range(200):
        mid = 0.5 * (lo + hi)
        if math.exp(mid - R) + LAM * mid > 0.0:
            hi = mid
        else:
            lo = mid
    return 0.5 * (lo + hi)

Z0 = _solve_z0()
assert Z0 > CLIP_MIN + 1e-6, "relu-fold requires Z0 > CLIP_MIN"

_CACHE = {}


def _build_nc():
    """Build (once) the single-core Bass/Tile program shared by all cores."""
    if "nc" in _CACHE:
        return _CACHE["nc"]

    from contextlib import ExitStack

    import concourse.bass as bass
    import concourse.mybir as mybir
    import concourse.tile as tile
    from concourse import bacc
    from concourse.masks import make_identity

    f16 = mybir.dt.float16
    f32 = mybir.dt.float32
    f32r = mybir.dt.float32r
    Alu = mybir.AluOpType
    Act = mybir.ActivationFunctionType

    nc = bacc.Bacc("TRN2", target_bir_lowering=False, debug=False, num_devices=N_CORES)

    blob_d = nc.dram_tensor("blob", [BLOB_ELEMS], f16, kind="ExternalInput")
    out_d = nc.dram_tensor("out", [S, DS], f16, kind="ExternalOutput")

    blob_t = blob_d.ap().tensor

    def bap(offset, ap):
        return bass.AP(tensor=blob_t, offset=offset, ap=[list(p) for p in ap])

    def r32(ap):
        return ap.bitcast(f32r)

    with tile.TileContext(nc) as tc, ExitStack() as ctx:
        # ---- x dedup: upload 512 rows/core, AllGather per batch quad ----
        dramp = ctx.enter_context(tc.tile_pool(name="dram", bufs=1, space="DRAM"))
        cc_in = dramp.tile([XROWS, D_MODEL], f16, tag="ccin")
        cc_out = dramp.tile([S, D_MODEL], f16, tag="ccout")
        nc.gpsimd.dma_start(
            out=cc_in[:], in_=bap(XOFF, [[D_MODEL, XROWS], [1, D_MODEL]])
        )
        nc.gpsimd.collective_compute(
            "AllGather",
            Alu.bypass,
            replica_groups=[[0, 1, 2, 3], [4, 5, 6, 7]],
            ins=[cc_in.opt()],
            outs=[cc_out.opt()],
        )

        persist = ctx.enter_context(tc.tile_pool(name="persist", bufs=1))

        ident = persist.tile([128, 128], f32, tag="ident")
        make_identity(nc, ident)

        cexp_sb = persist.tile([128, 1], f32, tag="cexp")
        nc.vector.memset(cexp_sb, C_EXP)

        # per-partition vectors: [128, 6] = dsc(2) | ab(2) | bb(2)
        vecs_raw = persist.tile([128, 6], f16, tag="vecsr")
        nc.sync.dma_start(out=vecs_raw, in_=bap(VOFF, [[1, 128], [128, 6]]))
        vecs_sb = persist.tile([128, 6], f32, tag="vecs")
        nc.vector.tensor_copy(vecs_sb, vecs_raw)
        dsc_sb = vecs_sb[:, 0:2]
        ab_sb = vecs_sb[:, 2:4]
        bb_sb = vecs_sb[:, 4:6]

        # broadcast d_v and (d_v*b) slices across partitions: [128, DS]
        dvb_raw = persist.tile([128, DS], f16, tag="dvbr")
        nc.sync.dma_start(out=dvb_raw, in_=bap(DVOFF, [[0, 128], [1, DS]]))
        dv_bc = persist.tile([128, DS], f32, tag="dvbc")
        nc.vector.tensor_copy(dv_bc, dvb_raw)
        bvb_raw = persist.tile([128, DS], f16, tag="bvbr")
        nc.sync.dma_start(out=bvb_raw, in_=bap(BVOFF, [[0, 128], [1, DS]]))
        bv_bc = persist.tile([128, DS], f32, tag="bvbc")
        nc.scalar.copy(bv_bc, bvb_raw)

        # W tiles: fp16 [128, DS] per k-tile -> f32 w_sb, wv_sb = w * d_v
        w_sb = persist.tile([128, KT, DS], f32r, tag="w")
        wv_sb = persist.tile([128, KT, DS], f32r, tag="wv")
        with tc.tile_pool(name="wraw", bufs=2) as wrp:
            for kt in range(KT):
                wraw = wrp.tile([128, DS], f16, tag="wr")
                nc.sync.dma_start(
                    out=wraw, in_=bap(WOFF + kt * 128 * DS, [[DS, 128], [1, DS]])
                )
                nc.scalar.copy(w_sb[:, kt, :], wraw)
                nc.vector.tensor_mul(
                    wv_sb[:, kt, :], w_sb[:, kt, :].bitcast(f32), dv_bc
                )

        # A = dsc * X_proj^T-slice (+dsc*b), B = X_proj^T-slice (+b): [128, 2, S]
        A_sb = persist.tile([128, 2, S], f32r, tag="A")
        B_sb = persist.tile([128, 2, S], f32r, tag="B")
        # V (+ones col) in natural layout: [128(t within tile), ST, 4*VW]
        V_sb = persist.tile([128, ST, HPC * VW], f32r, tag="V")
        for h in range(HPC):
            nc.vector.memset(V_sb[:, :, h * VW + HEAD_DIM].bitcast(f32), 1.0)

        # ---------------- phase 0: x^T, X_proj^T (A/B), V ----------------
        with tc.tile_pool(name="xT", bufs=1) as xtp, \
             tc.tile_pool(name="xraw", bufs=4) as xrp, \
             tc.tile_pool(name="xnat", bufs=8) as xnp, \
             tc.tile_pool(name="ptr", bufs=2, space="PSUM") as ptrp, \
             tc.tile_pool(name="pxp", bufs=2, space="PSUM") as pxpp, \
             tc.tile_pool(name="pv", bufs=2, space="PSUM") as pvp:
            xT = xtp.tile([128, KT, S], f32r)  # x^T: [k within tile, kt, s]

            for sg in range(4):  # groups of 512 s-rows
                xnat = []
                for j in range(4):
                    st = sg * 4 + j
                    xraw = xrp.tile([128, D_MODEL], f16, tag="xr", name=f"xr{sg}_{j}")
                    nc.sync.dma_start(
                        out=xraw, in_=cc_out[st * 128:(st + 1) * 128, :]
                    )
                    t = xnp.tile([128, D_MODEL], f32, tag="xn", name=f"xn{sg}_{j}")
                    if j % 2 == 0:
                        nc.vector.tensor_copy(t, xraw)
                    else:
                        nc.scalar.copy(t, xraw)
                    xnat.append(t)
                for ktg in range(4):  # pairs of k-tiles
                    ptr = ptrp.tile([128, 2, 512], f32, tag="ptr")
                    for i in range(2):
                        kt = ktg * 2 + i
                        for j in range(4):
                            nc.tensor.transpose(
                                ptr[:, i, j * 128:(j + 1) * 128],
                                xnat[j][:, kt * 128:(kt + 1) * 128],
                                ident,
                            )
                    for i in range(2):
                        kt = ktg * 2 + i
                        dst = xT[:, kt, sg * 512:(sg + 1) * 512]
                        if i == 0:
                            nc.scalar.copy(dst, ptr[:, i, :])
                        else:
                            nc.vector.tensor_copy(dst, ptr[:, i, :])

                # X_proj^T for this s-block: out rows = our 256 d-cols
                for dt in range(2):
                    pxp = pxpp.tile([128, 512], f32, tag="pxp")
                    for kt in range(KT):
                        nc.tensor.matmul(
                            pxp,
                            w_sb[:, kt, dt * 128:(dt + 1) * 128],
                            xT[:, kt, sg * 512:(sg + 1) * 512],
                            start=(kt == 0),
                            stop=(kt == KT - 1),
                        )
                    nc.scalar.activation(
                        A_sb[:, dt, sg * 512:(sg + 1) * 512], pxp,
                        Act.Identity, bias=ab_sb[:, dt:dt + 1],
                        scale=dsc_sb[:, dt:dt + 1],
                    )
                    nc.scalar.activation(
                        B_sb[:, dt, sg * 512:(sg + 1) * 512], pxp,
                        Act.Identity, bias=bb_sb[:, dt:dt + 1], scale=1.0,
                    )

                # V rows for this s-block (4 t-tiles)
                for j in range(4):
                    st = sg * 4 + j
                    pv = pvp.tile([128, DS], f32, tag="pv")
                    for kt in range(KT):
                        nc.tensor.matmul(
                            pv,
                            xT[:, kt, st * 128:(st + 1) * 128],
                            wv_sb[:, kt, :],
                            start=(kt == 0),
                            stop=(kt == KT - 1),
                        )
                    dst = V_sb[:, st, :].rearrange("p (h c) -> p h c", h=HPC)[:, :, 0:HEAD_DIM]
                    nc.vector.tensor_add(
                        dst,
                        pv.rearrange("p (h c) -> p h c", h=HPC),
                        bv_bc.rearrange("p (h c) -> p h c", h=HPC),
                    )

        # ---------------- main attention loop ----------------
        with tc.tile_pool(name="zp", bufs=2, space="PSUM") as zp, \
             tc.tile_pool(name="pop", bufs=2, space="PSUM") as pop, \
             tc.tile_pool(name="mp", bufs=6) as mp, \
             tc.tile_pool(name="ep", bufs=6) as ep, \
             tc.tile_pool(name="op", bufs=3) as op, \
             tc.tile_pool(name="outp", bufs=4) as outp, \
             tc.tile_pool(name="recp", bufs=4) as recp:
            for hp in range(2):        # head pair
                for sh in range(2):    # s-half (1024 query columns)
                    po = [pop.tile([VW, 1024], f32, tag="po", name=f"po{hp}_{sh}_{i}") for i in range(2)]
                    for tt in range(ST):
                        for h2 in range(2):
                            ha = hp * 2 + h2
                            dt, r0 = ha // 2, 64 * (ha % 2)
                            z_t = zp.tile([128, 1024], f32, tag="zslot")
                            for nb in range(2):
                                nc.tensor.matmul(
                                    z_t[:, nb * 512:(nb + 1) * 512],
                                    A_sb[r0:r0 + 64, dt, tt * 128:(tt + 1) * 128],
                                    B_sb[r0:r0 + 64, dt,
                                         sh * 1024 + nb * 512:sh * 1024 + (nb + 1) * 512],
                                    start=True, stop=True,
                                )
                            m_t = mp.tile([128, 1024], f32r, tag="m")
                            nc.vector.tensor_scalar(
                                out=m_t, in0=z_t,
                                scalar1=CLIP_MAX, scalar2=Z0,
                                op0=Alu.min, op1=Alu.max,
                            )
                            e_t = ep.tile([128, 1024], f32r, tag="e")
                            nc.scalar.activation(e_t, m_t.bitcast(f32), Act.Exp,
                                                 bias=cexp_sb[:, 0:1], scale=1.0)
                            for si, src in enumerate((e_t, m_t)):
                                for nb in range(2):
                                    nc.tensor.matmul(
                                        po[h2][:, nb * 512:(nb + 1) * 512],
                                        V_sb[:, tt, ha * VW:(ha + 1) * VW],
                                        src[:, nb * 512:(nb + 1) * 512],
                                        start=(tt == 0 and si == 0),
                                        stop=(tt == ST - 1 and si == 1),
                                    )
                    # finalize: transpose out^T -> natural, divide by row-sum
                    o_sb = []
                    for h2 in range(2):
                        t = op.tile([VW, 1024], f32, tag="o", name=f"o{hp}_{sh}_{h2}")
                        nc.scalar.copy(t, po[h2])
                        o_sb.append(t)
                    for st in range(8):
                        pon = zp.tile([128, 2 * VW], f32, tag="zslot")
                        rec = recp.tile([128, 2], f32, tag="rec")
                        out_sb = outp.tile([128, 128], f16, tag="out")
                        for h2 in range(2):
                            nc.tensor.transpose(
                                pon[:, h2 * VW:(h2 + 1) * VW],
                                o_sb[h2][:, st * 128:(st + 1) * 128],
                                ident[0:VW, 0:VW],
                            )
                        nc.vector.reciprocal(
                            rec,
                            pon.rearrange("p (h c) -> p h c", h=2)[:, :, HEAD_DIM],
                        )
                        nc.scalar.activation(
                            out_sb[:, 0:64], pon[:, 0:HEAD_DIM],
                            Act.Identity, bias=0.0, scale=rec[:, 0:1],
                        )
                        nc.vector.tensor_scalar(
                            out=out_sb[:, 64:128],
                            in0=pon[:, VW:VW + HEAD_DIM],
                            scalar1=rec[:, 1:2], scalar2=None,
                            op0=Alu.mult,
                        )
                        nc.sync.dma_start(
                            out=out_d[sh * 1024 + st * 128:sh * 1024 + (st + 1) * 128,
                                      hp * 128:(hp + 1) * 128],
                            in_=out_sb,
                        )

    nc.compile()
    _CACHE["nc"] = nc
    return nc


def _get_exec():
    """Build (once) the cached jitted PJRT executable + persistent buffers."""
    if "exec" in _CACHE:
        return _CACHE["exec"]

    import jax
    import jax.numpy as jnp
    from jax.sharding import Mesh, NamedSharding, PartitionSpec
    from jax.experimental.shard_map import shard_map

    import concourse.mybir as mybir
    from concourse.bass2jax import (
        _bass_exec_p,
        install_neuronx_cc_hook,
        partition_id_tensor,
    )

    nc = _build_nc()
    install_neuronx_cc_hook()
    assert nc.dbg_addr is None, "kernel must be built with debug=False"

    partition_name = nc.partition_id_tensor.name if nc.partition_id_tensor else None

    in_names = []
    out_names = []
    out_avals = []
    for alloc in nc.m.functions[0].allocations:
        if not isinstance(alloc, mybir.MemoryLocationSet):
            continue
        name = alloc.memorylocations[0].name
        if alloc.kind == "ExternalInput":
            if name != partition_name:
                in_names.append(name)
        elif alloc.kind == "ExternalOutput":
            out_names.append(name)
            out_avals.append(
                jax.core.ShapedArray(tuple(alloc.tensor_shape), mybir.dt.np(alloc.dtype))
            )
    assert in_names == ["blob"] and out_names == ["out"], (in_names, out_names)
    n_params = len(in_names)
    in_names_full = list(in_names) + list(out_names)
    if partition_name is not None:
        in_names_full.append(partition_name)

    def _body(*args):
        operands = list(args)
        if partition_name is not None:
            operands.append(partition_id_tensor())
        outs = _bass_exec_p.bind(
            *operands,
            out_avals=tuple(out_avals),
            in_names=tuple(in_names_full),
            out_names=tuple(out_names),
            lowering_input_output_aliases=(),
            sim_require_finite=True,
            sim_require_nnan=True,
            nc=nc,
        )
        return tuple(outs)

    devices = jax.devices()[:N_CORES]
    assert len(devices) == N_CORES
    mesh = Mesh(np.asarray(devices), ("core",))
    spec = PartitionSpec("core")
    n_outs = len(out_names)
    sharded = jax.jit(
        shard_map(
            _body,
            mesh=mesh,
            in_specs=(spec,) * (n_params + n_outs),
            out_specs=(spec,) * n_outs,
            check_rep=False,
        ),
        keep_unused=True,
    )

    # Persistent on-device stand-ins for the (donation-free) output operands.
    # The kernel writes every element of `out`, so their contents are unused.
    sh = NamedSharding(mesh, spec)
    dummy_out = jax.jit(
        lambda: jnp.zeros((N_CORES * S, DS), jnp.float16), out_shardings=sh
    )()
    dummy_out.block_until_ready()

    _CACHE["exec"] = (sharded, dummy_out)
    return _CACHE["exec"]


def _pack_blob(x, W, b, d_q, d_k, d_v):
    """Assemble the packed per-core fp16 input blob: [N_CORES * 770, 1024]."""
    x16 = np.asarray(x, dtype=np.float16)
    W16 = np.asarray(W, dtype=np.float16)
    b = np.asarray(b, dtype=np.float32)
    d_q = np.asarray(d_q, dtype=np.float32)
    d_k = np.asarray(d_k, dtype=np.float32)
    d_v = np.asarray(d_v, dtype=np.float32)

    dsc = (d_q * d_k).astype(np.float16)
    ab = (d_q * d_k * b).astype(np.float16)
    bb = b.astype(np.float16)
    dv = d_v.astype(np.float16)
    bv = (d_v * b).astype(np.float16)

    blob = np.empty((N_CORES, BLOB_ELEMS), dtype=np.float16)
    for c in range(N_CORES):
        bi = c // (N_CORES // B)
        g = c % (N_CORES // B)
        q = c % 4
        sl = slice(g * DS, (g + 1) * DS)
        blob[c, XOFF:XOFF + XROWS * D_MODEL] = x16[bi, q * XROWS:(q + 1) * XROWS].reshape(-1)
        blob[c, WOFF:WOFF + D_MODEL * DS] = W16[:, sl].reshape(-1)
        blob[c, VOFF:VOFF + DS] = dsc[sl]
        blob[c, VOFF + DS:VOFF + 2 * DS] = ab[sl]
        blob[c, VOFF + 2 * DS:VOFF + 3 * DS] = bb[sl]
        blob[c, DVOFF:DVOFF + DS] = dv[sl]
        blob[c, BVOFF:BVOFF + DS] = bv[sl]
    return blob.reshape(N_CORES * (BLOB_ELEMS // D_MODEL), D_MODEL)


def kernel(x, W, b, d_q, d_k, d_v):
    """Full-input entry point: shards across 8 NeuronCores, returns [B,S,D]."""
    sharded, dummy_out = _get_exec()

    blob = _pack_blob(x, W, b, d_q, d_k, d_v)
    (out_arr,) = sharded(blob, dummy_out)
    of = np.asarray(out_arr).reshape(N_CORES, S, DS)

    out = np.empty((B, S, D_MODEL), dtype=np.float32)
    for c in range(N_CORES):
        bi = c // (N_CORES // B)
        g = c % (N_CORES // B)
        out[bi, :, g * DS:(g + 1) * DS] = of[c]
    return out
